# revision 24
# baseline (speedup 1.0000x reference)
"""Causal self-attention (B=2, T=2048, C=1024, H=16) on 8 TRN2 NeuronCores.

Sharding: data-parallel over batch (2 groups of 4 cores) x tensor-parallel
over heads (4 heads per core). Each core computes qkv for its 4 heads,
causal flash-style attention, and a partial output projection
(y_heads @ w_proj rows). Host sums the 4 partial projections per batch and
adds b_proj.

Per-core device pipeline (fp16 data path, fp32 accumulation; the q/k path
runs in fp8e4 with DoubleRow perf mode — score magnitudes are O(0.1) so
fp8 quantization of q,k perturbs softmax weights by <1%):
  x supplied pre-transposed as xT fp16 [C-part, T] and fp8 (for q/k)
  qT/kT fp8 [32*h + d%32, dhalf, T] via DoubleRow matmuls over C (w cols
      permuted host-side so each head's d is split 0-31/32-63)
  v fp16 [T-part, 4 heads x 64] (+ones column for rowsums)
  S^T[k,q] per head = DoubleRow(kT, qT) over 2x32 d-tiles (causal blocks
      only; diagonal handled by a 0/1 multiplicative mask after exp)
  P = exp(S^T/32768) (ACT, PSUM->SBUF fp16; no max-subtraction: scores O(1))
  y'^T[d,q] (+rowsum row) = v_aug.T @ P (PSUM accum over k blocks)
  bias/normalize: pv += bv (x) rowsum (rank-1 PE matmul), yT = pv * (1/rs)
      broadcast via rank-1 PE matmul of the reciprocal row
  outT tile = yT.T @ wp -> out fp16 [T-part, C] partial, DMA to DRAM
qkv of chunk j+1 and proj of chunk j-1 are interleaved into the attention
kb loop so the PE keeps streaming while the scalar engine runs exp.
"""
import threading

import numpy as np

import concourse.bass as bass
import concourse.tile as tile
from concourse import bacc, mybir
from concourse.bass2jax import (
    _bass_exec_p,
    install_neuronx_cc_hook,
    partition_id_tensor,
)

N_CORES = 8
B, T, C, H = 2, 2048, 1024, 16
D = C // H            # 64
HL = 4                # heads per core
G = H // HL           # 4 head groups
SCALE_EXP = 1.0 / 32768.0   # 1/sqrt(C) / (32*32 fp8 weight scales)
F32 = mybir.dt.float32
F16 = mybir.dt.float16
FP8 = mybir.dt.float8e4
DR = mybir.MatmulPerfMode.DoubleRow


def build_nc():
    nc = bacc.Bacc("TRN2", target_bir_lowering=False, debug=False,
                   num_devices=N_CORES)
    x16 = nc.dram_tensor("x16", [128, 8, T], F16, kind="ExternalInput").ap()
    x8 = nc.dram_tensor("x8", [128, 8, T], FP8, kind="ExternalInput").ap()
    wq = nc.dram_tensor("wq", [128, 4, 2, 256], FP8, kind="ExternalInput").ap()
    wk = nc.dram_tensor("wk", [128, 4, 2, 256], FP8, kind="ExternalInput").ap()
    wv = nc.dram_tensor("wv", [128, 8, 256], F16, kind="ExternalInput").ap()
    bq = nc.dram_tensor("bq", [128, 2], F32, kind="ExternalInput").ap()
    bk = nc.dram_tensor("bk", [128, 2], F32, kind="ExternalInput").ap()
    bv = nc.dram_tensor("bv", [1, 256], F16, kind="ExternalInput").ap()
    wp = nc.dram_tensor("wp", [128, 2, C], F16, kind="ExternalInput").ap()
    mask = nc.dram_tensor("mask", [128, 128], F16, kind="ExternalInput").ap()
    out = nc.dram_tensor("out", [T, C], F16, kind="ExternalOutput").ap()

    Exp = mybir.ActivationFunctionType.Exp
    add = mybir.AluOpType.add
    mult = mybir.AluOpType.mult

    with tile.TileContext(nc) as tc:
        with tc.tile_pool(name="const", bufs=1) as cp, \
             tc.tile_pool(name="shps", bufs=2, space="PSUM") as shp, \
             tc.tile_pool(name="sT", bufs=2, space="PSUM") as stp, \
             tc.tile_pool(name="pvps", bufs=2, space="PSUM") as pvp, \
             tc.tile_pool(name="ptp", bufs=5) as ptp, \
             tc.tile_pool(name="rowp", bufs=4) as rowp, \
             tc.tile_pool(name="outp", bufs=3) as outp:

            # ---- persistent sbuf ----
            xt16 = cp.tile([128, 8, T], F16, tag="xt16")     # xT fp16 (v path)
            xt8 = cp.tile([128, 8, T], FP8, tag="xt8")       # xT fp8 (qk path)
            # q/k fp16, pair-stacked: [64*(h%2) + d, pair, t]
            qT = cp.tile([128, 2, T], F16, tag="qT")
            kT = cp.tile([128, 2, T], F16, tag="kT")
            va = cp.tile([128, 16, HL, D + 1], F16, tag="va")  # v + ones col
            yt = cp.tile([128, 2, T], F16, tag="yt")
            wq8 = cp.tile([128, 4, 2, 256], FP8, tag="wq8")
            wk8 = cp.tile([128, 4, 2, 256], FP8, tag="wk8")
            wv16 = cp.tile([128, 8, 256], F16, tag="wv16")
            wp16 = cp.tile([128, 2, C], F16, tag="wp16")
            bq_sb = cp.tile([128, 2], F32, tag="bq_sb")
            bk_sb = cp.tile([128, 2], F32, tag="bk_sb")
            bv_sb = cp.tile([1, 256], F16, tag="bv_sb")
            ones_sb = cp.tile([1, 128], F16, tag="ones_sb")
            mask_sb = cp.tile([128, 128], F16, tag="mask_sb")

            # ---- phase 0: loads (inputs pre-cast/pre-swizzled host-side) ----
            nc.gpsimd.dma_start(wq8[:], wq[:])
            nc.gpsimd.dma_start(wk8[:], wk[:])
            nc.gpsimd.dma_start(wv16[:], wv[:])
            nc.scalar.dma_start(bq_sb[:], bq[:])
            nc.scalar.dma_start(bk_sb[:], bk[:])
            nc.scalar.dma_start(bv_sb[:], bv[:])
            nc.scalar.dma_start(mask_sb[:], mask[:])
            nc.vector.memset(va[:, :, :, D:D + 1], 1.0)
            nc.vector.memset(ones_sb[:], 1.0)

            # xT supplied pre-transposed; load per T-chunk so chunk 0 is
            # available almost immediately (fp8 first: q/k matmuls go first)
            for r in range(4):
                rs = slice(512 * r, 512 * (r + 1))
                nc.sync.dma_start(xt8[:, :, rs], x8[:, :, rs])
                nc.sync.dma_start(xt16[:, :, rs], x16[:, :, rs])
            nc.gpsimd.dma_start(wp16[:], wp[:])

            # ---- qkv building blocks (emitted as interleavable units) ----
            def qk_unit(j, w8, bias_sb, dst, p):
                # one DoubleRow matmul group: head pair p of q or k for
                # T-chunk j (fp8 inputs, 256-deep contraction per matmul)
                # -> dst[:, p, qs] fp16
                qs = slice(512 * j, 512 * (j + 1))
                ps = shp.tile([128, 512], F32, tag="sh")
                for kp in range(4):
                    nc.tensor.matmul(
                        ps[:],
                        w8[:, kp, :, 128 * p:128 * (p + 1)],
                        xt8[:, 2 * kp:2 * kp + 2, qs],
                        start=(kp == 0), stop=(kp == 3), perf_mode=DR)
                nc.vector.tensor_scalar_add(dst[:, p, qs], ps[:],
                                            bias_sb[:, p:p + 1])

            def v_unit(t):
                # v rows for one 128-t block (all 4 heads), fp16; the last
                # rank-1 matmul folds in the v bias (v = x@wv + bv exactly)
                psv = shp.tile([128, 256], F32, tag="sh")
                for kt_i in range(8):
                    nc.tensor.matmul(
                        psv[:],
                        xt16[:, kt_i, 128 * t:128 * (t + 1)],
                        wv16[:, kt_i, :],
                        start=(kt_i == 0), stop=False)
                nc.tensor.matmul(psv[:], ones_sb[:], bv_sb[:],
                                 start=False, stop=True)
                nc.vector.tensor_copy(
                    out=va[:, t, :, 0:D],
                    in_=psv[:].rearrange("p (h d) -> p h d", h=HL))

            def qkv_units(j):
                us = []
                for (w8, bias_sb, dst) in ((wq8, bq_sb, qT), (wk8, bk_sb, kT)):
                    for p in range(2):
                        us.append(lambda j=j, w8=w8, b=bias_sb, d=dst, p=p:
                                  qk_unit(j, w8, b, d, p))
                for t in range(4 * j, 4 * (j + 1)):
                    us.append(lambda t=t: v_unit(t))
                return us

            # ---- projection units ----
            def proj_unit(t, cc, osb):
                ops = shp.tile([128, 512], F32, tag="sh")
                for u in range(2):
                    nc.tensor.matmul(
                        ops[:],
                        yt[:, u, 128 * t:128 * (t + 1)],
                        wp16[:, u, 512 * cc:512 * (cc + 1)],
                        start=(u == 0), stop=(u == 1))
                # PSUM->SBUF fp16 cast, alternating engines to balance load
                eng = nc.vector if (t + cc) % 2 else nc.scalar
                if eng is nc.vector:
                    eng.tensor_copy(out=osb[:, 512 * cc:512 * (cc + 1)],
                                    in_=ops[:])
                else:
                    eng.copy(osb[:, 512 * cc:512 * (cc + 1)], ops[:])
                nc.gpsimd.dma_start(
                    out[128 * t:128 * (t + 1), 512 * cc:512 * (cc + 1)],
                    osb[:, 512 * cc:512 * (cc + 1)])

            def proj_units(jm1):
                us = []
                for t in range(4 * jm1, 4 * (jm1 + 1)):
                    osb = outp.tile([128, C], F16, tag="osb",
                                    name=f"osb{t}")
                    for cc in range(2):
                        us.append(lambda t=t, cc=cc, osb=osb:
                                  proj_unit(t, cc, osb))
                return us

            # ---- attention pair (heads 2p, 2p+1) for q-chunk j ----
            def attn_pair(p, j, fillers, stride, slot):
                pvs = [pvp.tile([65, 512], F32, tag="pv", name=f"pv{_hh}")
                       for _hh in range(2)]
                nkb = 4 * j + 4
                pending = None  # software pipeline: PV trails S/exp by 1
                for kb in range(nkb):
                    off = 128 * (kb - 4 * j) if kb >= 4 * j else 0
                    s2 = stp.tile([128, 2, 512], F32, tag="sT")
                    for hh in range(2):
                        pr = 64 * hh
                        nc.tensor.matmul(
                            s2[:, hh, off:512],
                            kT[pr:pr + 64, p, 128 * kb:128 * (kb + 1)],
                            qT[pr:pr + 64, p,
                               512 * j + off:512 * (j + 1)],
                            start=True, stop=True)
                    ptt = ptp.tile([128, 2, 512], F16, tag="pt")
                    nc.scalar.activation(ptt[:, :, off:512], s2[:, :, off:512],
                                         Exp, scale=SCALE_EXP)
                    if kb >= 4 * j:
                        # causal 0/1 mask on the diagonal block, post-exp
                        # (all-SBUF fp16, so it can run on gpsimd)
                        nc.gpsimd.tensor_tensor(
                            ptt[:, :, off:off + 128],
                            ptt[:, :, off:off + 128],
                            mask_sb[:, None, :].to_broadcast([128, 2, 128]),
                            mult)
                    if pending is not None:
                        pkb, poff, pptt = pending
                        for hh in range(2):
                            nc.tensor.matmul(
                                pvs[hh][:, poff:512],
                                va[:, pkb, 2 * p + hh, :],
                                pptt[:, hh, poff:512],
                                start=(pkb == 0), stop=False)
                    pending = (kb, off, ptt)
                    slot[0] += 1
                    if fillers and slot[0] % stride == 0:
                        fillers.pop(0)()
                pkb, poff, pptt = pending
                for hh in range(2):
                    nc.tensor.matmul(
                        pvs[hh][:, poff:512],
                        va[:, pkb, 2 * p + hh, :],
                        pptt[:, hh, poff:512],
                        start=(pkb == 0), stop=True)
                # normalize: yT = pv * (1/rowsum); the rowsum row is
                # broadcast to 64 partitions by a rank-1 PE matmul, then
                # reciprocal'd on DVE (avoids gpsimd custom-op lib loads)
                for hh in range(2):
                    pv = pvs[hh]
                    rowf = rowp.tile([1, 512], F16, tag="row")
                    nc.vector.tensor_copy(out=rowf[:], in_=pv[64:65, :])
                    rsb = shp.tile([128, 512], F32, tag="sh")
                    nc.tensor.matmul(rsb[0:64, :], ones_sb[:, 0:64], rowf[:],
                                     start=True, stop=True)
                    rec = rowp.tile([64, 512], F32, tag="rec")
                    nc.vector.reciprocal_approx_fast(rec[:], rsb[0:64, :])
                    nc.vector.tensor_tensor(
                        yt[64 * hh:64 * (hh + 1), p, 512 * j:512 * (j + 1)],
                        pv[0:64, :], rec[:], mult)

            # ---- interleaved pipeline over T-chunks ----
            for u in qkv_units(0):
                u()
            for j in range(4):
                fillers = []
                if j < 3:
                    fillers += qkv_units(j + 1)
                if j >= 1:
                    fillers += proj_units(j - 1)
                slots = 2 * (4 * j + 4)
                stride = max(1, slots // max(1, len(fillers)))
                slot = [0]
                for p in range(2):
                    attn_pair(p, j, fillers, stride, slot)
                for f in fillers:
                    f()
            for u in proj_units(3):
                u()

    nc.compile()
    return nc


def make_fn(nc):
    """Sharded 8-core jit callable for the compiled Bass program."""
    import jax
    from jax.sharding import Mesh, PartitionSpec
    from jax.experimental.shard_map import shard_map

    install_neuronx_cc_hook()
    in_names, out_names, out_avals, zero_outs = [], [], [], []
    pname = nc.partition_id_tensor.name if nc.partition_id_tensor else None
    for alloc in nc.m.functions[0].allocations:
        if not isinstance(alloc, mybir.MemoryLocationSet):
            continue
        name = alloc.memorylocations[0].name
        if alloc.kind == "ExternalInput":
            if name != pname:
                in_names.append(name)
        elif alloc.kind == "ExternalOutput":
            out_names.append(name)
            shape = tuple(alloc.tensor_shape)
            dtype = mybir.dt.np(alloc.dtype)
            out_avals.append(jax.core.ShapedArray(shape, dtype))
            zero_outs.append(np.zeros(shape, dtype))
    n_params = len(in_names)
    all_names = list(in_names) + out_names
    if pname is not None:
        all_names.append(pname)

    def _body(*args):
        operands = list(args)
        if pname is not None:
            operands.append(partition_id_tensor())
        outs = _bass_exec_p.bind(
            *operands, out_avals=tuple(out_avals), in_names=tuple(all_names),
            out_names=tuple(out_names), lowering_input_output_aliases=(),
            sim_require_finite=True, sim_require_nnan=True, nc=nc)
        return tuple(outs)

    devices = jax.devices()[:N_CORES]
    mesh = Mesh(np.asarray(devices), ("core",))
    n_out = len(out_names)
    fn = jax.jit(
        shard_map(_body, mesh=mesh,
                  in_specs=(PartitionSpec("core"),) * (n_params + n_out),
                  out_specs=(PartitionSpec("core"),) * n_out,
                  check_rep=False),
        keep_unused=True)
    return fn, in_names, out_names, zero_outs


def shard_inputs(x, w_attn, b_attn, w_proj, b_proj):
    """Build the per-core input maps (core = 4*batch + head_group).

    Host-side prep is layout only: slicing per core, fp16/fp8 rounding
    (q/k weights pre-scaled by 32 for fp8 range; folded back out via the
    exp scale), and the partition swizzles the device matmuls consume."""
    import ml_dtypes
    fp8 = ml_dtypes.float8_e4m3
    f16 = np.float16
    x = np.asarray(x, dtype=np.float32)
    w_attn = np.asarray(w_attn, dtype=np.float32)
    b_attn = np.asarray(b_attn, dtype=np.float32)
    w_proj = np.asarray(w_proj, dtype=np.float32)
    mask = np.where(np.arange(128)[None, :] >= np.arange(128)[:, None],
                    np.float16(1.0), np.float16(0.0))
    # [C, n] -> [128, C//128, n] partition swizzle
    swz = lambda w, dt: np.ascontiguousarray(
        w.reshape(-1, 128, w.shape[1]).transpose(1, 0, 2).astype(dt))

    # q/k weights: DoubleRow layout [128, 4 ktile-pairs, 2, 256]
    def qk_swz(w):  # w [1024, 256] -> fp8 DR layout, pre-scaled by 32
        return np.ascontiguousarray(
            (32.0 * w).reshape(4, 2, 128, 256).transpose(2, 0, 1, 3)
        ).astype(fp8)

    x16 = [np.ascontiguousarray(
        x[b].T.reshape(8, 128, T).transpose(1, 0, 2)) for b in range(B)]
    in_maps = []
    for core in range(N_CORES):
        b, g = divmod(core, G)
        cs = slice(256 * g, 256 * (g + 1))
        bqk = lambda v: np.ascontiguousarray(
            (32.0 * v).reshape(2, 128).T.astype(np.float32))
        in_maps.append({
            "x16": x16[b].astype(f16),
            "x8": x16[b].astype(fp8),
            "wq": qk_swz(w_attn[:, cs]),
            "wk": qk_swz(w_attn[:, 1024:][:, cs]),
            "wv": swz(w_attn[:, 2048:][:, cs], f16),
            "bq": bqk(b_attn[cs]),
            "bk": bqk(b_attn[1024:][cs]),
            "bv": np.ascontiguousarray(b_attn[2048:][cs].astype(f16))[None],
            "wp": swz(w_proj[cs, :], f16),
            "mask": mask,
        })
    return in_maps


_cache = {}
_lock = threading.Lock()


def _get_compiled():
    with _lock:
        if "fn" not in _cache:
            nc = build_nc()
            fn, in_names, out_names, zero_outs = make_fn(nc)
            _cache.update(fn=fn, nc=nc, in_names=in_names,
                          out_names=out_names, zero_outs=zero_outs)
    return _cache


def run_cores(in_maps):
    """Execute the 8-core SPMD program, return per-core output dicts."""
    import jax

    cc = _get_compiled()
    concat_in = [np.concatenate([m[k] for m in in_maps], axis=0)
                 for k in cc["in_names"]]
    concat_zeros = [np.zeros((N_CORES * z.shape[0], *z.shape[1:]), z.dtype)
                    for z in cc["zero_outs"]]
    outs = cc["fn"](*[jax.device_put(v) for v in concat_in],
                    *[jax.device_put(z) for z in concat_zeros])
    res = []
    for c in range(N_CORES):
        res.append({name: np.asarray(outs[i]).reshape(
            N_CORES, *cc["zero_outs"][i].shape)[c]
            for i, name in enumerate(cc["out_names"])})
    return res


def kernel(x, w_attn, b_attn, w_proj, b_proj):
    in_maps = shard_inputs(x, w_attn, b_attn, w_proj, b_proj)
    res = run_cores(in_maps)
    b_proj = np.asarray(b_proj, dtype=np.float32)
    out = np.empty((B, T, C), dtype=np.float32)
    for b in range(B):
        acc = res[4 * b]["out"].astype(np.float32)
        for g in range(1, G):
            acc = acc + res[4 * b + g]["out"].astype(np.float32)
        out[b] = acc + b_proj
    return out


# revision 31
# speedup vs baseline: 1.0319x; 1.0319x over previous
"""Causal self-attention (B=2, T=2048, C=1024, H=16) on 8 TRN2 NeuronCores.

Sharding: data-parallel over batch (2 groups of 4 cores) x tensor-parallel
over heads (4 heads per core). Each core computes qkv for its 4 heads,
causal flash-style attention, and a partial output projection
(y_heads @ w_proj rows). Host sums the 4 partial projections per batch and
adds b_proj.

Per-core device pipeline (fp16 data path, fp32 accumulation; the q/k path
runs in fp8e4 with DoubleRow perf mode — score magnitudes are O(0.1) so
fp8 quantization of q,k perturbs softmax weights by <1%):
  x supplied pre-transposed as xT fp16 [C-part, T] and fp8 (for q/k)
  qT/kT fp8 [32*h + d%32, dhalf, T] via DoubleRow matmuls over C (w cols
      permuted host-side so each head's d is split 0-31/32-63)
  v fp16 [T-part, 4 heads x 64] (+ones column for rowsums)
  S^T[k,q] per head = DoubleRow(kT, qT) over 2x32 d-tiles (causal blocks
      only; diagonal handled by a 0/1 multiplicative mask after exp)
  P = exp(S^T/32768) (ACT, PSUM->SBUF fp16; no max-subtraction: scores O(1))
  y'^T[d,q] (+rowsum row) = v_aug.T @ P (PSUM accum over k blocks)
  bias/normalize: pv += bv (x) rowsum (rank-1 PE matmul), yT = pv * (1/rs)
      broadcast via rank-1 PE matmul of the reciprocal row
  outT tile = yT.T @ wp -> out fp16 [T-part, C] partial, DMA to DRAM
qkv of chunk j+1 and proj of chunk j-1 are interleaved into the attention
kb loop so the PE keeps streaming while the scalar engine runs exp.
"""
import threading

import numpy as np

import concourse.bass as bass
import concourse.tile as tile
from concourse import bacc, mybir
from concourse.bass2jax import (
    _bass_exec_p,
    install_neuronx_cc_hook,
    partition_id_tensor,
)

N_CORES = 8
B, T, C, H = 2, 2048, 1024, 16
D = C // H            # 64
HL = 4                # heads per core
G = H // HL           # 4 head groups
SCALE_EXP = 1.0 / 32768.0   # 1/sqrt(C) / (32*32 fp8 weight scales)
F32 = mybir.dt.float32
F16 = mybir.dt.float16
FP8 = mybir.dt.float8e4
DR = mybir.MatmulPerfMode.DoubleRow


def build_nc():
    nc = bacc.Bacc("TRN2", target_bir_lowering=False, debug=False,
                   num_devices=N_CORES)
    # x chunk-major so each 512-t chunk is one contiguous DMA
    x16 = nc.dram_tensor("x16", [128, 4, 8, 512], F16, kind="ExternalInput").ap()
    x8 = nc.dram_tensor("x8", [128, 4, 8, 512], FP8, kind="ExternalInput").ap()
    wq = nc.dram_tensor("wq", [128, 4, 2, 256], FP8, kind="ExternalInput").ap()
    wk = nc.dram_tensor("wk", [128, 4, 2, 256], FP8, kind="ExternalInput").ap()
    wv = nc.dram_tensor("wv", [128, 8, 256], F16, kind="ExternalInput").ap()
    bq = nc.dram_tensor("bq", [128, 2], F32, kind="ExternalInput").ap()
    bk = nc.dram_tensor("bk", [128, 2], F32, kind="ExternalInput").ap()
    bv = nc.dram_tensor("bv", [1, 256], F16, kind="ExternalInput").ap()
    wp = nc.dram_tensor("wp", [128, 2, C], F16, kind="ExternalInput").ap()
    mask = nc.dram_tensor("mask", [128, 128], F16, kind="ExternalInput").ap()
    out = nc.dram_tensor("out", [T, C], F16, kind="ExternalOutput").ap()

    Exp = mybir.ActivationFunctionType.Exp
    add = mybir.AluOpType.add
    mult = mybir.AluOpType.mult

    with tile.TileContext(nc) as tc:
        with tc.tile_pool(name="const", bufs=1) as cp, \
             tc.tile_pool(name="shps", bufs=2, space="PSUM") as shp, \
             tc.tile_pool(name="sT", bufs=2, space="PSUM") as stp, \
             tc.tile_pool(name="pvps", bufs=2, space="PSUM") as pvp, \
             tc.tile_pool(name="ptp", bufs=5) as ptp, \
             tc.tile_pool(name="rowp", bufs=4) as rowp, \
             tc.tile_pool(name="outp", bufs=3) as outp:

            # ---- persistent sbuf ----
            xt16 = cp.tile([128, 4, 8, 512], F16, tag="xt16")  # xT f16 (v path)
            xt8 = cp.tile([128, 4, 8, 512], FP8, tag="xt8")    # xT fp8 (qk)
            # q/k fp16, pair-stacked: [64*(h%2) + d, pair, t]
            qT = cp.tile([128, 2, T], F16, tag="qT")
            kT = cp.tile([128, 2, T], F16, tag="kT")
            va = cp.tile([128, 16, HL, D + 1], F16, tag="va")  # v + ones col
            yt = cp.tile([128, 2, T], F16, tag="yt")
            wq8 = cp.tile([128, 4, 2, 256], FP8, tag="wq8")
            wk8 = cp.tile([128, 4, 2, 256], FP8, tag="wk8")
            wv16 = cp.tile([128, 8, 256], F16, tag="wv16")
            wp16 = cp.tile([128, 2, C], F16, tag="wp16")
            bq_sb = cp.tile([128, 2], F32, tag="bq_sb")
            bk_sb = cp.tile([128, 2], F32, tag="bk_sb")
            bv_sb = cp.tile([1, 256], F16, tag="bv_sb")
            ones_sb = cp.tile([1, 128], F16, tag="ones_sb")
            mask_sb = cp.tile([128, 128], F16, tag="mask_sb")

            # ---- phase 0: loads (inputs pre-cast/pre-swizzled host-side) ----
            nc.gpsimd.dma_start(wq8[:], wq[:])
            nc.gpsimd.dma_start(wk8[:], wk[:])
            nc.gpsimd.dma_start(wv16[:], wv[:])
            nc.scalar.dma_start(bq_sb[:], bq[:])
            nc.scalar.dma_start(bk_sb[:], bk[:])
            nc.scalar.dma_start(bv_sb[:], bv[:])
            nc.scalar.dma_start(mask_sb[:], mask[:])
            nc.vector.memset(va[:, :, :, D:D + 1], 1.0)
            nc.vector.memset(ones_sb[:], 1.0)

            # xT supplied pre-transposed; load per T-chunk so chunk 0 is
            # available almost immediately (fp8 first: q/k matmuls go first)
            for r in range(4):
                nc.sync.dma_start(xt8[:, r], x8[:, r])
                nc.sync.dma_start(xt16[:, r], x16[:, r])
            nc.gpsimd.dma_start(wp16[:], wp[:])

            # ---- qkv building blocks (emitted as interleavable units) ----
            def qk_unit(j, w8, bias_sb, dst, p):
                # one DoubleRow matmul group: head pair p of q or k for
                # T-chunk j (fp8 inputs, 256-deep contraction per matmul)
                # -> dst[:, p, qs] fp16
                qs = slice(512 * j, 512 * (j + 1))
                ps = shp.tile([128, 512], F32, tag="sh")
                for kp in range(4):
                    nc.tensor.matmul(
                        ps[:],
                        w8[:, kp, :, 128 * p:128 * (p + 1)],
                        xt8[:, j, 2 * kp:2 * kp + 2, :],
                        start=(kp == 0), stop=(kp == 3), perf_mode=DR)
                nc.vector.tensor_scalar_add(dst[:, p, qs], ps[:],
                                            bias_sb[:, p:p + 1])

            def v_unit(t):
                # v rows for one 128-t block (all 4 heads), fp16; the last
                # rank-1 matmul folds in the v bias (v = x@wv + bv exactly)
                psv = shp.tile([128, 256], F32, tag="sh")
                for kt_i in range(8):
                    nc.tensor.matmul(
                        psv[:],
                        xt16[:, t // 4, kt_i, 128 * (t % 4):128 * (t % 4 + 1)],
                        wv16[:, kt_i, :],
                        start=(kt_i == 0), stop=False)
                nc.tensor.matmul(psv[:], ones_sb[:], bv_sb[:],
                                 start=False, stop=True)
                nc.vector.tensor_copy(
                    out=va[:, t, :, 0:D],
                    in_=psv[:].rearrange("p (h d) -> p h d", h=HL))

            def qkv_units(j):
                us = []
                for (w8, bias_sb, dst) in ((wq8, bq_sb, qT), (wk8, bk_sb, kT)):
                    for p in range(2):
                        us.append(lambda j=j, w8=w8, b=bias_sb, d=dst, p=p:
                                  qk_unit(j, w8, b, d, p))
                for t in range(4 * j, 4 * (j + 1)):
                    us.append(lambda t=t: v_unit(t))
                return us

            # ---- projection units ----
            def proj_unit(t, cc, osb):
                ops = shp.tile([128, 512], F32, tag="sh")
                for u in range(2):
                    nc.tensor.matmul(
                        ops[:],
                        yt[:, u, 128 * t:128 * (t + 1)],
                        wp16[:, u, 512 * cc:512 * (cc + 1)],
                        start=(u == 0), stop=(u == 1))
                # PSUM->SBUF fp16 cast, alternating engines to balance load
                if (t + cc) % 2:
                    nc.vector.tensor_copy(out=osb[:, 512 * cc:512 * (cc + 1)],
                                          in_=ops[:])
                else:
                    nc.scalar.copy(osb[:, 512 * cc:512 * (cc + 1)], ops[:])
                if cc == 1:
                    # one contiguous [128, C] DMA per tile (2KB rows)
                    deng = nc.gpsimd if t % 2 else nc.sync
                    deng.dma_start(out[128 * t:128 * (t + 1), :], osb[:])

            def proj_units(jm1):
                us = []
                for t in range(4 * jm1, 4 * (jm1 + 1)):
                    osb = outp.tile([128, C], F16, tag="osb",
                                    name=f"osb{t}")
                    for cc in range(2):
                        us.append(lambda t=t, cc=cc, osb=osb:
                                  proj_unit(t, cc, osb))
                return us

            # ---- attention pair (heads 2p, 2p+1) for q-chunk j ----
            def attn_pair(p, j, fillers, stride, slot):
                pvs = [pvp.tile([65, 512], F32, tag="pv", name=f"pv{_hh}")
                       for _hh in range(2)]
                nkb = 4 * j + 4
                pending = None  # software pipeline: PV trails S/exp by 1
                for kb in range(nkb):
                    off = 128 * (kb - 4 * j) if kb >= 4 * j else 0
                    s2 = stp.tile([128, 2, 512], F32, tag="sT")
                    for hh in range(2):
                        pr = 64 * hh
                        nc.tensor.matmul(
                            s2[:, hh, off:512],
                            kT[pr:pr + 64, p, 128 * kb:128 * (kb + 1)],
                            qT[pr:pr + 64, p,
                               512 * j + off:512 * (j + 1)],
                            start=True, stop=True)
                    ptt = ptp.tile([128, 2, 512], F16, tag="pt")
                    nc.scalar.activation(ptt[:, :, off:512], s2[:, :, off:512],
                                         Exp, scale=SCALE_EXP)
                    if kb >= 4 * j:
                        # causal 0/1 mask on the diagonal block, post-exp
                        # (all-SBUF fp16, so it can run on gpsimd)
                        nc.gpsimd.tensor_tensor(
                            ptt[:, :, off:off + 128],
                            ptt[:, :, off:off + 128],
                            mask_sb[:, None, :].to_broadcast([128, 2, 128]),
                            mult)
                    if pending is not None:
                        pkb, poff, pptt = pending
                        for hh in range(2):
                            nc.tensor.matmul(
                                pvs[hh][:, poff:512],
                                va[:, pkb, 2 * p + hh, :],
                                pptt[:, hh, poff:512],
                                start=(pkb == 0), stop=False)
                    pending = (kb, off, ptt)
                    slot[0] += 1
                    if fillers and slot[0] % stride == 0:
                        fillers.pop(0)()
                pkb, poff, pptt = pending
                for hh in range(2):
                    nc.tensor.matmul(
                        pvs[hh][:, poff:512],
                        va[:, pkb, 2 * p + hh, :],
                        pptt[:, hh, poff:512],
                        start=(pkb == 0), stop=True)
                # normalize: yT = pv * (1/rowsum); the rowsum row is
                # broadcast to 64 partitions by a rank-1 PE matmul, then
                # reciprocal'd on DVE (avoids gpsimd custom-op lib loads)
                for hh in range(2):
                    pv = pvs[hh]
                    rowf = rowp.tile([1, 512], F16, tag="row")
                    nc.vector.tensor_copy(out=rowf[:], in_=pv[64:65, :])
                    rsb = shp.tile([128, 512], F32, tag="sh")
                    nc.tensor.matmul(rsb[0:64, :], ones_sb[:, 0:64], rowf[:],
                                     start=True, stop=True)
                    rec = rowp.tile([64, 512], F32, tag="rec")
                    nc.vector.reciprocal_approx_fast(rec[:], rsb[0:64, :])
                    nc.vector.tensor_tensor(
                        yt[64 * hh:64 * (hh + 1), p, 512 * j:512 * (j + 1)],
                        pv[0:64, :], rec[:], mult)

            # ---- interleaved pipeline over T-chunks ----
            for u in qkv_units(0):
                u()
            for j in range(4):
                fillers = []
                if j < 3:
                    fillers += qkv_units(j + 1)
                if j >= 1:
                    fillers += proj_units(j - 1)
                slots = 2 * (4 * j + 4)
                stride = max(1, slots // max(1, len(fillers)))
                slot = [0]
                for p in range(2):
                    attn_pair(p, j, fillers, stride, slot)
                for f in fillers:
                    f()
            for u in proj_units(3):
                u()

    nc.compile()
    return nc


def make_fn(nc):
    """Sharded 8-core jit callable for the compiled Bass program."""
    import jax
    from jax.sharding import Mesh, PartitionSpec
    from jax.experimental.shard_map import shard_map

    install_neuronx_cc_hook()
    in_names, out_names, out_avals, zero_outs = [], [], [], []
    pname = nc.partition_id_tensor.name if nc.partition_id_tensor else None
    for alloc in nc.m.functions[0].allocations:
        if not isinstance(alloc, mybir.MemoryLocationSet):
            continue
        name = alloc.memorylocations[0].name
        if alloc.kind == "ExternalInput":
            if name != pname:
                in_names.append(name)
        elif alloc.kind == "ExternalOutput":
            out_names.append(name)
            shape = tuple(alloc.tensor_shape)
            dtype = mybir.dt.np(alloc.dtype)
            out_avals.append(jax.core.ShapedArray(shape, dtype))
            zero_outs.append(np.zeros(shape, dtype))
    n_params = len(in_names)
    all_names = list(in_names) + out_names
    if pname is not None:
        all_names.append(pname)

    def _body(*args):
        operands = list(args)
        if pname is not None:
            operands.append(partition_id_tensor())
        outs = _bass_exec_p.bind(
            *operands, out_avals=tuple(out_avals), in_names=tuple(all_names),
            out_names=tuple(out_names), lowering_input_output_aliases=(),
            sim_require_finite=True, sim_require_nnan=True, nc=nc)
        return tuple(outs)

    devices = jax.devices()[:N_CORES]
    mesh = Mesh(np.asarray(devices), ("core",))
    n_out = len(out_names)
    fn = jax.jit(
        shard_map(_body, mesh=mesh,
                  in_specs=(PartitionSpec("core"),) * (n_params + n_out),
                  out_specs=(PartitionSpec("core"),) * n_out,
                  check_rep=False),
        keep_unused=True)
    return fn, in_names, out_names, zero_outs


def shard_inputs(x, w_attn, b_attn, w_proj, b_proj):
    """Build the per-core input maps (core = 4*batch + head_group).

    Host-side prep is layout only: slicing per core, fp16/fp8 rounding
    (q/k weights pre-scaled by 32 for fp8 range; folded back out via the
    exp scale), and the partition swizzles the device matmuls consume."""
    import ml_dtypes
    fp8 = ml_dtypes.float8_e4m3
    f16 = np.float16
    x = np.asarray(x, dtype=np.float32)
    w_attn = np.asarray(w_attn, dtype=np.float32)
    b_attn = np.asarray(b_attn, dtype=np.float32)
    w_proj = np.asarray(w_proj, dtype=np.float32)
    mask = np.where(np.arange(128)[None, :] >= np.arange(128)[:, None],
                    np.float16(1.0), np.float16(0.0))
    # [C, n] -> [128, C//128, n] partition swizzle
    swz = lambda w, dt: np.ascontiguousarray(
        w.reshape(-1, 128, w.shape[1]).transpose(1, 0, 2).astype(dt))

    # q/k weights: DoubleRow layout [128, 4 ktile-pairs, 2, 256]
    def qk_swz(w):  # w [1024, 256] -> fp8 DR layout, pre-scaled by 32
        return np.ascontiguousarray(
            (32.0 * w).reshape(4, 2, 128, 256).transpose(2, 0, 1, 3)
        ).astype(fp8)

    # [T, C] -> [128, 4 chunk, 8 ktile, 512] (chunk-major, contiguous DMAs)
    x16 = [np.ascontiguousarray(
        x[b].T.reshape(8, 128, 4, 512).transpose(1, 2, 0, 3))
        for b in range(B)]
    in_maps = []
    for core in range(N_CORES):
        b, g = divmod(core, G)
        cs = slice(256 * g, 256 * (g + 1))
        bqk = lambda v: np.ascontiguousarray(
            (32.0 * v).reshape(2, 128).T.astype(np.float32))
        in_maps.append({
            "x16": x16[b].astype(f16),
            "x8": x16[b].astype(fp8),
            "wq": qk_swz(w_attn[:, cs]),
            "wk": qk_swz(w_attn[:, 1024:][:, cs]),
            "wv": swz(w_attn[:, 2048:][:, cs], f16),
            "bq": bqk(b_attn[cs]),
            "bk": bqk(b_attn[1024:][cs]),
            "bv": np.ascontiguousarray(b_attn[2048:][cs].astype(f16))[None],
            "wp": swz(w_proj[cs, :], f16),
            "mask": mask,
        })
    return in_maps


_cache = {}
_lock = threading.Lock()


def _get_compiled():
    with _lock:
        if "fn" not in _cache:
            nc = build_nc()
            fn, in_names, out_names, zero_outs = make_fn(nc)
            _cache.update(fn=fn, nc=nc, in_names=in_names,
                          out_names=out_names, zero_outs=zero_outs)
    return _cache


def run_cores(in_maps):
    """Execute the 8-core SPMD program, return per-core output dicts."""
    import jax

    cc = _get_compiled()
    concat_in = [np.concatenate([m[k] for m in in_maps], axis=0)
                 for k in cc["in_names"]]
    concat_zeros = [np.zeros((N_CORES * z.shape[0], *z.shape[1:]), z.dtype)
                    for z in cc["zero_outs"]]
    outs = cc["fn"](*[jax.device_put(v) for v in concat_in],
                    *[jax.device_put(z) for z in concat_zeros])
    res = []
    for c in range(N_CORES):
        res.append({name: np.asarray(outs[i]).reshape(
            N_CORES, *cc["zero_outs"][i].shape)[c]
            for i, name in enumerate(cc["out_names"])})
    return res


def kernel(x, w_attn, b_attn, w_proj, b_proj):
    in_maps = shard_inputs(x, w_attn, b_attn, w_proj, b_proj)
    res = run_cores(in_maps)
    b_proj = np.asarray(b_proj, dtype=np.float32)
    out = np.empty((B, T, C), dtype=np.float32)
    for b in range(B):
        acc = res[4 * b]["out"].astype(np.float32)
        for g in range(1, G):
            acc = acc + res[4 * b + g]["out"].astype(np.float32)
        out[b] = acc + b_proj
    return out
